# revision 5
# baseline (speedup 1.0000x reference)
"""BYOL loss kernel for Trainium2 (8 NeuronCores, SPMD data-parallel).

v15: raw bass (no TileContext), earliest-possible supply, ACT-PSUM scratch.

loss = 2 - 2 * mean_n( <x_n, t_n> / (||x_n|| * ||t_n||) )   N=8192, D=512.

Per core (1024 rows = 8 blocks of [128, 512], fp8 e3m4, x/t interleaved
per block in one DRAM tensor):
- Sync issues the 8 supply DMAs immediately after its engine preamble
  (no Tile prologue barrier), each completion bumping `supply` by 16.
- ACT: warmup Square (pulls ACT_TABLE_LOAD into the preamble window), then
  xx for all 8 blocks + tt for blocks 0-2, PSUM scratch, accum to `dots`.
- DVE: xt for all 8 blocks + tt for blocks 3-7, SBUF scratch.
- Each engine waits supply >= 16*(i+1) before block i's first op.
- Products of blocks 0-6 bump done_main (21), block 7's bump done_b7 (3);
  the two output DMAs wait on those, then Sync waits for their completion.
- Host: cos + mean in f64 from the per-row (xt, xx, tt) sums.
"""

import sys

for _p in ("/opt/trn_rl_repo",):
    if _p not in sys.path:
        sys.path.insert(0, _p)

import ml_dtypes
import numpy as np

from concourse import bacc, mybir
from concourse import bass_utils

N, D = 8192, 512
NCORES = 8
N_LOC = N // NCORES
P = 128
NT = N_LOC // P

F32 = mybir.dt.float32
BF16 = mybir.dt.bfloat16
FP8 = mybir.dt.float8e3
MULT = mybir.AluOpType.mult

ACT_TT = {0, 1, 2}   # tt blocks handled by ACT (plus xx of every block)


def _build():
    nc = bacc.Bacc("TRN2", target_bir_lowering=False, debug=False, num_devices=NCORES)
    xt_in = nc.dram_tensor("xt", [P, 2 * NT * D], FP8, kind="ExternalInput").ap()
    out = nc.dram_tensor("dots", [P, 3 * NT], F32, kind="ExternalOutput").ap()
    off = {"xt": 0, "xx": 1, "tt": 2}

    # One semaphore PER supply DMA: a dma_start's completion arrives as 16
    # independent +1 incs (one per SDMA engine), so a single shared counter
    # at threshold 16*i can trip while an earlier transfer is still partly
    # in flight. Per-transfer sems at threshold 16 are unambiguous.
    supply = [nc.alloc_semaphore(f"supply_{i}") for i in range(NT)]
    done_main = nc.alloc_semaphore("done_main")
    done_b7 = nc.alloc_semaphore("done_b7")
    outsem = nc.alloc_semaphore("outsem")
    # Raw-bass kernels skip TileContext's sem hygiene: clear our sems up
    # front so a prior NEFF's leftover values can never satisfy a wait
    # early. GpSimd is idle in this window; first inc lands >2us later.
    for s in (*supply, done_main, done_b7, outsem):
        nc.gpsimd.sem_clear(s)

    raw_in = nc.alloc_sbuf_tensor("raw_in", [P, 2 * NT * D], FP8)
    warm = nc.alloc_sbuf_tensor("warm_raw", [P, 1], BF16)
    dots = nc.alloc_sbuf_tensor("dots_sb", [P, 3 * NT], F32)
    prs = [nc.alloc_sbuf_tensor(f"pr{k}", [P, D], BF16) for k in range(2)]
    sqs = [nc.alloc_psum_tensor(f"sq{k}", [P, D], F32) for k in range(2)]

    one_bf16 = nc.const_aps.aps[(BF16, 1.0)]
    nc.scalar.activation(warm.ap(), one_bf16, mybir.ActivationFunctionType.Square)

    for i in range(NT):
        nc.sync.dma_start(
            raw_in.ap()[:, i * 2 * D : (i + 1) * 2 * D],
            xt_in[:, i * 2 * D : (i + 1) * 2 * D],
        ).then_inc(supply[i], 16)

    def acc_ap(stat, i):
        c = 3 * i + off[stat]
        return dots.ap()[:, c : c + 1]

    def dsem(i):
        return done_b7 if i == NT - 1 else done_main

    n_act = [0]
    n_dve = [0]

    def act_square(src, stat, i):
        sq = sqs[n_act[0] % 2]
        n_act[0] += 1
        nc.scalar.activation(
            sq.ap(), src, mybir.ActivationFunctionType.Square,
            accum_out=acc_ap(stat, i),
        ).then_inc(dsem(i), 1)

    def dve_stt(a, b, stat, i):
        pr = prs[n_dve[0] % 2]
        n_dve[0] += 1
        nc.vector.scalar_tensor_tensor(
            pr.ap(), a, 1.0, b, op0=MULT, op1=MULT,
            accum_out=acc_ap(stat, i),
        ).then_inc(dsem(i), 1)

    for i in range(NT):
        xa = raw_in.ap()[:, (2 * i) * D : (2 * i + 1) * D]
        ta = raw_in.ap()[:, (2 * i + 1) * D : (2 * i + 2) * D]
        nc.scalar.wait_ge(supply[i], 16)
        nc.vector.wait_ge(supply[i], 16)
        act_square(xa, "xx", i)
        if i in ACT_TT:
            act_square(ta, "tt", i)
        else:
            dve_stt(ta, ta, "tt", i)
        dve_stt(xa, ta, "xt", i)

    assert n_act[0] + n_dve[0] == 3 * NT

    nc.sync.wait_ge(done_main, 3 * (NT - 1))
    nc.sync.dma_start(out[:, : 3 * (NT - 1)], dots.ap()[:, : 3 * (NT - 1)]).then_inc(
        outsem, 16
    )
    nc.sync.wait_ge(done_b7, 3)
    nc.sync.dma_start(out[:, 3 * (NT - 1) :], dots.ap()[:, 3 * (NT - 1) :]).then_inc(
        outsem, 16
    )
    nc.sync.wait_ge(outsem, 32)

    nc.finalize()
    return nc


_nc_cache = None


def _get_nc():
    global _nc_cache
    if _nc_cache is None:
        _nc_cache = _build()
    return _nc_cache


def run(x, x_target, **spmd_kwargs):
    nc = _get_nc()
    x = np.asarray(x, dtype=np.float32).astype(ml_dtypes.float8_e3m4)
    t = np.asarray(x_target, dtype=np.float32).astype(ml_dtypes.float8_e3m4)
    assert x.shape == (N, D) and t.shape == (N, D)
    in_maps = []
    for c in range(NCORES):
        xs = x[c * N_LOC : (c + 1) * N_LOC].reshape(P, NT, 1, D)
        ts = t[c * N_LOC : (c + 1) * N_LOC].reshape(P, NT, 1, D)
        pair = np.concatenate([xs, ts], axis=2).reshape(P, 2 * NT * D)
        in_maps.append({"xt": np.ascontiguousarray(pair)})
    res = bass_utils.run_bass_kernel_spmd(
        nc, in_maps, core_ids=list(range(NCORES)), **spmd_kwargs
    )
    dots = np.stack([np.asarray(r["dots"]) for r in res.results]).astype(np.float64)
    dots = dots.reshape(NCORES, P, NT, 3)
    xt = dots[..., 0]
    xx = dots[..., 1]
    tt = dots[..., 2]
    EPS = 1e-8
    cos = xt / (np.maximum(np.sqrt(xx), EPS) * np.maximum(np.sqrt(tt), EPS))
    loss = 2.0 - 2.0 * float(np.mean(cos))
    return np.float32(loss), res


def kernel(x, x_target):
    loss, _ = run(x, x_target)
    return loss


# revision 6
# speedup vs baseline: 1.0012x; 1.0012x over previous
"""BYOL loss kernel for Trainium2 (8 NeuronCores, SPMD data-parallel).

v15: raw bass (no TileContext), earliest-possible supply, ACT-PSUM scratch.

loss = 2 - 2 * mean_n( <x_n, t_n> / (||x_n|| * ||t_n||) )   N=8192, D=512.

Per core (1024 rows = 8 blocks of [128, 512], fp8 e3m4, x/t interleaved
per block in one DRAM tensor):
- Sync issues the 8 supply DMAs immediately after its engine preamble
  (no Tile prologue barrier), each completing on its OWN semaphore
  (a dma_start completion is 16 independent +1 incs, one per SDMA engine,
  so a shared counter with 16*i thresholds races; per-transfer sems don't).
- ACT: warmup Square (pulls ACT_TABLE_LOAD into the preamble window), then
  xx for all 8 blocks + tt for blocks 0-2, PSUM scratch, accum to `dots`.
- DVE: xt for all 8 blocks + tt for blocks 3-7, SBUF scratch.
- Each engine waits supply[i] >= 16 before block i's first op.
- Products of blocks 0-6 bump done_main (21), block 7's bump done_b7 (3);
  the two output DMAs wait on those, then Sync waits for their completion.
- Host: cos + mean in f64 from the per-row (xt, xx, tt) sums.

Measured (nominal p-state): ~21.7us vs 24.5us for the v11 Tile baseline.
The residual profile is ~3.9us preamble+supply latency, ~8.9us balanced
ACT/DVE streams (the accum granularity of [P,1] per op makes 24 ops the
floor; STT/ACTIVATE run 1x per element for every dtype on this silicon),
~1.6us output-DMA completion, ~7.3us fixed walrus postamble rounds.
"""

import sys

for _p in ("/opt/trn_rl_repo",):
    if _p not in sys.path:
        sys.path.insert(0, _p)

import ml_dtypes
import numpy as np

from concourse import bacc, mybir
from concourse import bass_utils

N, D = 8192, 512
NCORES = 8
N_LOC = N // NCORES
P = 128
NT = N_LOC // P

F32 = mybir.dt.float32
BF16 = mybir.dt.bfloat16
FP8 = mybir.dt.float8e3
MULT = mybir.AluOpType.mult

ACT_TT = {0, 1, 2}   # tt blocks handled by ACT (plus xx of every block)


def _build():
    nc = bacc.Bacc("TRN2", target_bir_lowering=False, debug=False, num_devices=NCORES)
    xt_in = nc.dram_tensor("xt", [P, 2 * NT * D], FP8, kind="ExternalInput").ap()
    out = nc.dram_tensor("dots", [P, 3 * NT], F32, kind="ExternalOutput").ap()
    off = {"xt": 0, "xx": 1, "tt": 2}

    # One semaphore PER supply DMA: a dma_start's completion arrives as 16
    # independent +1 incs (one per SDMA engine), so a single shared counter
    # at threshold 16*i can trip while an earlier transfer is still partly
    # in flight. Per-transfer sems at threshold 16 are unambiguous.
    supply = [nc.alloc_semaphore(f"supply_{i}") for i in range(NT)]
    done_main = nc.alloc_semaphore("done_main")
    done_b7 = nc.alloc_semaphore("done_b7")
    outsem = nc.alloc_semaphore("outsem")
    # Raw-bass kernels skip TileContext's sem hygiene: clear our sems up
    # front so a prior NEFF's leftover values can never satisfy a wait
    # early. GpSimd is idle in this window; first inc lands >2us later.
    for s in (*supply, done_main, done_b7, outsem):
        nc.gpsimd.sem_clear(s)

    raw_in = nc.alloc_sbuf_tensor("raw_in", [P, 2 * NT * D], FP8)
    warm = nc.alloc_sbuf_tensor("warm_raw", [P, 1], BF16)
    dots = nc.alloc_sbuf_tensor("dots_sb", [P, 3 * NT], F32)
    prs = [nc.alloc_sbuf_tensor(f"pr{k}", [P, D], BF16) for k in range(2)]
    sqs = [nc.alloc_psum_tensor(f"sq{k}", [P, D], F32) for k in range(2)]

    one_bf16 = nc.const_aps.aps[(BF16, 1.0)]
    nc.scalar.activation(warm.ap(), one_bf16, mybir.ActivationFunctionType.Square)

    for i in range(NT):
        nc.sync.dma_start(
            raw_in.ap()[:, i * 2 * D : (i + 1) * 2 * D],
            xt_in[:, i * 2 * D : (i + 1) * 2 * D],
        ).then_inc(supply[i], 16)

    def acc_ap(stat, i):
        c = 3 * i + off[stat]
        return dots.ap()[:, c : c + 1]

    def dsem(i):
        return done_b7 if i == NT - 1 else done_main

    n_act = [0]
    n_dve = [0]

    def act_square(src, stat, i):
        sq = sqs[n_act[0] % 2]
        n_act[0] += 1
        nc.scalar.activation(
            sq.ap(), src, mybir.ActivationFunctionType.Square,
            accum_out=acc_ap(stat, i),
        ).then_inc(dsem(i), 1)

    def dve_stt(a, b, stat, i):
        pr = prs[n_dve[0] % 2]
        n_dve[0] += 1
        nc.vector.scalar_tensor_tensor(
            pr.ap(), a, 1.0, b, op0=MULT, op1=MULT,
            accum_out=acc_ap(stat, i),
        ).then_inc(dsem(i), 1)

    for i in range(NT):
        xa = raw_in.ap()[:, (2 * i) * D : (2 * i + 1) * D]
        ta = raw_in.ap()[:, (2 * i + 1) * D : (2 * i + 2) * D]
        nc.scalar.wait_ge(supply[i], 16)
        nc.vector.wait_ge(supply[i], 16)
        act_square(xa, "xx", i)
        if i in ACT_TT:
            act_square(ta, "tt", i)
        else:
            dve_stt(ta, ta, "tt", i)
        dve_stt(xa, ta, "xt", i)

    assert n_act[0] + n_dve[0] == 3 * NT

    nc.sync.wait_ge(done_main, 3 * (NT - 1))
    nc.sync.dma_start(out[:, : 3 * (NT - 1)], dots.ap()[:, : 3 * (NT - 1)]).then_inc(
        outsem, 16
    )
    nc.sync.wait_ge(done_b7, 3)
    nc.sync.dma_start(out[:, 3 * (NT - 1) :], dots.ap()[:, 3 * (NT - 1) :]).then_inc(
        outsem, 16
    )
    nc.sync.wait_ge(outsem, 32)

    nc.finalize()
    return nc


_nc_cache = None


def _get_nc():
    global _nc_cache
    if _nc_cache is None:
        _nc_cache = _build()
    return _nc_cache


def run(x, x_target, **spmd_kwargs):
    nc = _get_nc()
    x = np.asarray(x, dtype=np.float32).astype(ml_dtypes.float8_e3m4)
    t = np.asarray(x_target, dtype=np.float32).astype(ml_dtypes.float8_e3m4)
    assert x.shape == (N, D) and t.shape == (N, D)
    in_maps = []
    for c in range(NCORES):
        xs = x[c * N_LOC : (c + 1) * N_LOC].reshape(P, NT, 1, D)
        ts = t[c * N_LOC : (c + 1) * N_LOC].reshape(P, NT, 1, D)
        pair = np.concatenate([xs, ts], axis=2).reshape(P, 2 * NT * D)
        in_maps.append({"xt": np.ascontiguousarray(pair)})
    res = bass_utils.run_bass_kernel_spmd(
        nc, in_maps, core_ids=list(range(NCORES)), **spmd_kwargs
    )
    dots = np.stack([np.asarray(r["dots"]) for r in res.results]).astype(np.float64)
    dots = dots.reshape(NCORES, P, NT, 3)
    xt = dots[..., 0]
    xx = dots[..., 1]
    tt = dots[..., 2]
    EPS = 1e-8
    cos = xt / (np.maximum(np.sqrt(xx), EPS) * np.maximum(np.sqrt(tt), EPS))
    loss = 2.0 - 2.0 * float(np.mean(cos))
    return np.float32(loss), res


def kernel(x, x_target):
    loss, _ = run(x, x_target)
    return loss
